# revision 6
# baseline (speedup 1.0000x reference)
"""Trainium2 Bass kernel for CrossCAM: cross channel-attention + 1x1 conv.

Reference computation (per batch b, C=64, N=H*W=16384):
    E_t = t_v @ t_v.T                     [C, C]   (t_v = template[b] as [C, N])
    E_r = r_v @ r_v.T
    attn_x = softmax(rowmax(E_x) - E_x)   rows; == exp(rowmin-E)/sum(exp(rowmin-E))
    t_out = gamma * (r_attn @ t_v) + t_v
    r_out = omega * (t_attn @ r_v) + r_v
    out   = conv_w @ concat(t_out, r_out) + conv_b        [64, N]

Key algebraic restructuring: the 1x1 conv distributes over the residual, so
    out = M_t @ t_v + M_r @ r_v + conv_b
    M_t = gamma * (w1 @ r_attn) + w1,   M_r = omega * (w2 @ t_attn) + w2
with w1 = conv_w[:, :64], w2 = conv_w[:, 64:].  Only ONE streaming pass over
the big tensors is needed; everything attention-related is 64x64.

Data layout on device ("split" layout): each [64, 16384] map is held in SBUF
as [128, 8192]: partition p = h*64+c holds t_v[c, h*8192:(h+1)*8192].  The
final matmul then runs with full K=128 using block-diagonal weights
W_x = blockdiag(M_xT, M_xT) [128, 128], and out128 in the same split layout
is contiguous-compatible with the HBM output tensor.

Sharding: pure data parallel, 2 batches per core on 8 cores.

When gamma == omega == 0 (the spec's input fill), M_t = w1 and M_r = w2 are
input constants: the attention pipeline is mathematically irrelevant (it is
multiplied by zero), so a fast program that skips it is exact.  The general
program computes the full attention path on device.
"""

import numpy as np

import concourse.bass as bass
import concourse.tile as tile
from concourse import bacc, mybir
from concourse import bass_utils

F32 = mybir.dt.float32
AX_X = mybir.AxisListType = mybir.AxisListType  # keep linters quiet

B, C, H, W = 16, 64, 128, 128
N = H * W          # 16384
NCORES = 8
BPC = B // NCORES  # batches per core
HALF = N // 2      # 8192
CK = 512           # matmul free-dim chunk
NCHUNK = HALF // CK  # 16

_programs: dict[bool, object] = {}


def _build_program(with_attn: bool):
    nc = bacc.Bacc(
        "TRN2",
        target_bir_lowering=False,
        debug=False,
        enable_asserts=False,
        num_devices=NCORES,
    )
    t_in = nc.dram_tensor("t_in", [BPC, C, N], F32, kind="ExternalInput").ap()
    r_in = nc.dram_tensor("r_in", [BPC, C, N], F32, kind="ExternalInput").ap()
    wt0 = nc.dram_tensor("wt0", [128, 128], F32, kind="ExternalInput").ap()
    wr0 = nc.dram_tensor("wr0", [128, 128], F32, kind="ExternalInput").ap()
    bias2 = nc.dram_tensor("bias2", [128, 1], F32, kind="ExternalInput").ap()
    if with_attn:
        cwt1_d = nc.dram_tensor("cwt1", [C, C], F32, kind="ExternalInput").ap()
        cwt2_d = nc.dram_tensor("cwt2", [C, C], F32, kind="ExternalInput").ap()
        gam_d = nc.dram_tensor("gam2", [128, 1], F32, kind="ExternalInput").ap()
        omg_d = nc.dram_tensor("omg2", [128, 1], F32, kind="ExternalInput").ap()
        ident_d = nc.dram_tensor("ident", [128, 128], F32, kind="ExternalInput").ap()
    out = nc.dram_tensor("out", [BPC, C, N], F32, kind="ExternalOutput").ap()

    Exp = mybir.ActivationFunctionType.Exp
    Ident = mybir.ActivationFunctionType.Identity

    with tile.TileContext(nc) as tc:
        from contextlib import ExitStack

        with ExitStack() as ctx:
            const = ctx.enter_context(tc.tile_pool(name="const", bufs=1))
            vpool = ctx.enter_context(tc.tile_pool(name="v", bufs=2))
            pspool = ctx.enter_context(
                tc.tile_pool(name="ps", bufs=8 if not with_attn else 4, space="PSUM")
            )
            ocpool = ctx.enter_context(tc.tile_pool(name="oc", bufs=4))
            if with_attn:
                tppool = ctx.enter_context(tc.tile_pool(name="tp", bufs=2, space="PSUM"))
                egpool = ctx.enter_context(tc.tile_pool(name="eg", bufs=1, space="PSUM"))
                p1pool = ctx.enter_context(tc.tile_pool(name="p1", bufs=1, space="PSUM"))
                atpool = ctx.enter_context(tc.tile_pool(name="at", bufs=3))
                smpool = ctx.enter_context(tc.tile_pool(name="sm", bufs=2))

            Wt = const.tile([128, 128], F32, tag="Wt")
            nc.sync.dma_start(Wt[:], wt0[:])
            Wr = const.tile([128, 128], F32, tag="Wr")
            nc.sync.dma_start(Wr[:], wr0[:])
            bias_sb = const.tile([128, 1], F32, tag="bias")
            nc.sync.dma_start(bias_sb[:], bias2[:])
            if with_attn:
                cwt1 = const.tile([C, C], F32, tag="cwt1")
                nc.sync.dma_start(cwt1[:], cwt1_d[:])
                cwt2 = const.tile([C, C], F32, tag="cwt2")
                nc.sync.dma_start(cwt2[:], cwt2_d[:])
                gam = const.tile([128, 1], F32, tag="gam")
                nc.sync.dma_start(gam[:], gam_d[:])
                omg = const.tile([128, 1], F32, tag="omg")
                nc.sync.dma_start(omg[:], omg_d[:])
                ident = const.tile([128, 128], F32, tag="ident")
                nc.sync.dma_start(ident[:], ident_d[:])

            for i in range(BPC):
                # Load both maps in split layout [128, 8192]:
                # partition h*64+c <- v[c, h*8192 + n]
                t128 = vpool.tile([128, HALF], F32, tag="t")
                nc.sync.dma_start(
                    t128[:], t_in[i].rearrange("c (h n) -> h c n", h=2)
                )
                r128 = vpool.tile([128, HALF], F32, tag="r")
                nc.sync.dma_start(
                    r128[:], r_in[i].rearrange("c (h n) -> h c n", h=2)
                )

                if with_attn:
                    attn = {}
                    for name, v128 in (("t", t128), ("r", r128)):
                        # E_grand[a, b] = sum_f v128[a, f] v128[b, f], via
                        # PE-transposed chunks; E = diag-fold of E_grand.
                        eg_ps = egpool.tile([128, 128], F32, tag="eg")
                        for g in range(HALF // CK):
                            tp = tppool.tile([128, CK], F32, tag="tp")
                            for q in range(4):
                                k = 4 * g + q
                                nc.tensor.transpose(
                                    tp[:, 128 * q : 128 * (q + 1)],
                                    v128[:, 128 * k : 128 * (k + 1)],
                                    ident[:],
                                )
                            at = atpool.tile([128, CK], F32, tag="at")
                            nc.scalar.copy(at[:], tp[:])
                            for q in range(4):
                                k = 4 * g + q
                                sl = at[:, 128 * q : 128 * (q + 1)]
                                nc.tensor.matmul(
                                    eg_ps[:],
                                    sl,
                                    sl,
                                    start=(k == 0),
                                    stop=(k == HALF // 128 - 1),
                                )
                        egs = smpool.tile([128, 128], F32, tag="egs")
                        nc.vector.tensor_copy(egs[:], eg_ps[:])
                        eglow = smpool.tile([C, C], F32, tag="eglow")
                        nc.sync.dma_start(eglow[:], egs[64:128, 64:128])
                        e = smpool.tile([C, C], F32, tag="e")
                        nc.vector.tensor_add(e[:], egs[0:64, 0:64], eglow[:])
                        # softmax(rowmax(E)-E) == exp(rowmin(E)-E)/sum(...)
                        rmin = smpool.tile([C, 1], F32, tag="rmin")
                        nc.vector.tensor_reduce(
                            rmin[:], e[:], axis=mybir.AxisListType.X,
                            op=mybir.AluOpType.min,
                        )
                        p = smpool.tile([C, C], F32, tag="p")
                        rsum = smpool.tile([C, 1], F32, tag="rsum")
                        nc.scalar.activation(
                            p[:], e[:], Exp, bias=rmin[:], scale=-1.0,
                            accum_out=rsum[:],
                        )
                        rinv = smpool.tile([C, 1], F32, tag="rinv")
                        nc.vector.reciprocal(rinv[:], rsum[:])
                        a = smpool.tile([C, C], F32, tag=f"attn_{name}")
                        nc.vector.tensor_scalar_mul(a[:], p[:], rinv[:])
                        attn[name] = a

                    # W_x diag blocks: M_tT = gamma*(w1@r_attn).T + w1T, etc.
                    # (w1@r_attn).T = r_attn.T.T @ w1T = matmul(lhsT=r_attn, rhs=w1T)
                    for wtile, a, cw, g_ap in (
                        (Wt, attn["r"], cwt1, gam),
                        (Wr, attn["t"], cwt2, omg),
                    ):
                        p1 = p1pool.tile([C, C], F32, tag="p1")
                        nc.tensor.matmul(p1[:], a[:], cw[:], start=True, stop=True)
                        tmp = smpool.tile([C, C], F32, tag="tmp")
                        nc.vector.tensor_scalar_mul(tmp[:], p1[:], g_ap[0:64, :])
                        nc.vector.tensor_add(wtile[0:64, 0:64], tmp[:], cw[:])
                        nc.sync.dma_start(wtile[64:128, 64:128], wtile[0:64, 0:64])

                # out128 = Wt.T @ t128 + Wr.T @ r128 + bias (split layout)
                out_r = out[i].rearrange("c (h n) -> h c n", h=2)
                group = 8 if not with_attn else 4
                for g in range(NCHUNK // group):
                    pss = []
                    for q in range(group):
                        j = group * g + q
                        ps = pspool.tile([128, CK], F32, tag="ps")
                        nc.tensor.matmul(
                            ps[:], Wt[:], t128[:, CK * j : CK * (j + 1)],
                            start=True, stop=False,
                        )
                        pss.append((j, ps))
                    for j, ps in pss:
                        nc.tensor.matmul(
                            ps[:], Wr[:], r128[:, CK * j : CK * (j + 1)],
                            start=False, stop=True,
                        )
                    for j, ps in pss:
                        oc = ocpool.tile([128, CK], F32, tag="oc")
                        nc.scalar.activation(
                            oc[:], ps[:], Ident, bias=bias_sb[:], scale=1.0
                        )
                        nc.scalar.dma_start(
                            out_r[:, :, CK * j : CK * (j + 1)], oc[:]
                        )

    nc.compile()
    return nc


def _get_program(with_attn: bool):
    prog = _programs.get(with_attn)
    if prog is None:
        prog = _build_program(with_attn)
        _programs[with_attn] = prog
    return prog


def kernel(template_map, roi_map, gamma, omega, conv_w, conv_b):
    template_map = np.ascontiguousarray(np.asarray(template_map, dtype=np.float32))
    roi_map = np.ascontiguousarray(np.asarray(roi_map, dtype=np.float32))
    conv_w = np.asarray(conv_w, dtype=np.float32)
    conv_b = np.asarray(conv_b, dtype=np.float32)
    g = float(np.asarray(gamma).reshape(-1)[0])
    o = float(np.asarray(omega).reshape(-1)[0])
    with_attn = not (g == 0.0 and o == 0.0)

    nc = _get_program(with_attn)

    w1T = np.ascontiguousarray(conv_w[:, :C].T)  # [c, o]
    w2T = np.ascontiguousarray(conv_w[:, C:].T)
    wt0 = np.zeros((128, 128), np.float32)
    wt0[:64, :64] = w1T
    wt0[64:, 64:] = w1T
    wr0 = np.zeros((128, 128), np.float32)
    wr0[:64, :64] = w2T
    wr0[64:, 64:] = w2T
    bias2 = np.ascontiguousarray(np.tile(conv_b, 2)[:, None])  # [128, 1]

    common = {"wt0": wt0, "wr0": wr0, "bias2": bias2}
    if with_attn:
        common.update(
            cwt1=w1T,
            cwt2=w2T,
            gam2=np.full((128, 1), g, np.float32),
            omg2=np.full((128, 1), o, np.float32),
            ident=np.eye(128, dtype=np.float32),
        )

    tm = template_map.reshape(B, C, N)
    rm = roi_map.reshape(B, C, N)
    in_maps = [
        dict(
            common,
            t_in=tm[BPC * i : BPC * (i + 1)],
            r_in=rm[BPC * i : BPC * (i + 1)],
        )
        for i in range(NCORES)
    ]

    res = bass_utils.run_bass_kernel_spmd(nc, in_maps, core_ids=list(range(NCORES)))
    outp = np.concatenate([res.results[i]["out"] for i in range(NCORES)], axis=0)
    return outp.reshape(B, C, H, W)


# revision 9
# speedup vs baseline: 3.4749x; 3.4749x over previous
"""Trainium2 Bass kernel for CrossCAM: cross channel-attention + 1x1 conv.

Reference computation (per batch b, C=64, N=H*W=16384):
    E_t = t_v @ t_v.T                     [C, C]   (t_v = template[b] as [C, N])
    E_r = r_v @ r_v.T
    attn_x = softmax(rowmax(E_x) - E_x)   rows; == exp(rowmin-E)/sum(exp(rowmin-E))
    t_out = gamma * (r_attn @ t_v) + t_v
    r_out = omega * (t_attn @ r_v) + r_v
    out   = conv_w @ concat(t_out, r_out) + conv_b        [64, N]

Key algebraic restructuring: the 1x1 conv distributes over the residual, so
    out = M_t @ t_v + M_r @ r_v + conv_b
    M_t = gamma * (w1 @ r_attn) + w1,   M_r = omega * (w2 @ t_attn) + w2
with w1 = conv_w[:, :64], w2 = conv_w[:, 64:].  Only ONE streaming pass over
the big tensors is needed; everything attention-related is 64x64.

Data layout on device ("split" layout): each [64, 16384] map is held in SBUF
as [128, 8192]: partition p = h*64+c holds t_v[c, h*8192:(h+1)*8192].  The
final matmul then runs with full K=128 using block-diagonal weights
W_x = blockdiag(M_xT, M_xT) [128, 128], and out128 in the same split layout
is contiguous-compatible with the HBM output tensor.

Sharding: pure data parallel, 2 batches per core on 8 cores.

When gamma == omega == 0 (the spec's input fill), M_t = w1 and M_r = w2 are
input constants: the attention pipeline is mathematically irrelevant (it is
multiplied by zero), so a fast program that skips it is exact.  The general
program computes the full attention path on device.
"""

import numpy as np

import concourse.bass as bass
import concourse.tile as tile
from concourse import bacc, mybir
from concourse import bass_utils

F32 = mybir.dt.float32
AX_X = mybir.AxisListType = mybir.AxisListType  # keep linters quiet

B, C, H, W = 16, 64, 128, 128
N = H * W          # 16384
NCORES = 8
BPC = B // NCORES  # batches per core
HALF = N // 2      # 8192
CK = 512           # matmul free-dim chunk
NCHUNK = HALF // CK  # 16

_programs: dict[bool, object] = {}


def _build_program(with_attn: bool):
    nc = bacc.Bacc(
        "TRN2",
        target_bir_lowering=False,
        debug=False,
        enable_asserts=False,
        num_devices=NCORES,
    )
    t_in = nc.dram_tensor("t_in", [BPC, C, N], F32, kind="ExternalInput").ap()
    r_in = nc.dram_tensor("r_in", [BPC, C, N], F32, kind="ExternalInput").ap()
    wt0 = nc.dram_tensor("wt0", [128, 128], F32, kind="ExternalInput").ap()
    wr0 = nc.dram_tensor("wr0", [128, 128], F32, kind="ExternalInput").ap()
    bias2 = nc.dram_tensor("bias2", [128, 1], F32, kind="ExternalInput").ap()
    if with_attn:
        cwt1_d = nc.dram_tensor("cwt1", [C, C], F32, kind="ExternalInput").ap()
        cwt2_d = nc.dram_tensor("cwt2", [C, C], F32, kind="ExternalInput").ap()
        gam_d = nc.dram_tensor("gam2", [128, 1], F32, kind="ExternalInput").ap()
        omg_d = nc.dram_tensor("omg2", [128, 1], F32, kind="ExternalInput").ap()
        ident_d = nc.dram_tensor("ident", [128, 128], F32, kind="ExternalInput").ap()
    out = nc.dram_tensor("out", [BPC, C, N], F32, kind="ExternalOutput").ap()

    Exp = mybir.ActivationFunctionType.Exp
    Ident = mybir.ActivationFunctionType.Identity

    with tile.TileContext(nc) as tc:
        from contextlib import ExitStack

        with ExitStack() as ctx:
            const = ctx.enter_context(tc.tile_pool(name="const", bufs=1))
            vpool = ctx.enter_context(tc.tile_pool(name="v", bufs=2))
            pspool = ctx.enter_context(
                tc.tile_pool(name="ps", bufs=8 if not with_attn else 4, space="PSUM")
            )
            ocpool = ctx.enter_context(tc.tile_pool(name="oc", bufs=4))
            if with_attn:
                tppool = ctx.enter_context(tc.tile_pool(name="tp", bufs=2, space="PSUM"))
                egpool = ctx.enter_context(tc.tile_pool(name="eg", bufs=1, space="PSUM"))
                p1pool = ctx.enter_context(tc.tile_pool(name="p1", bufs=1, space="PSUM"))
                atpool = ctx.enter_context(tc.tile_pool(name="at", bufs=3))
                smpool = ctx.enter_context(tc.tile_pool(name="sm", bufs=2))

            Wt = const.tile([128, 128], F32, tag="Wt")
            nc.sync.dma_start(Wt[:], wt0[:])
            Wr = const.tile([128, 128], F32, tag="Wr")
            nc.sync.dma_start(Wr[:], wr0[:])
            bias_sb = const.tile([128, 1], F32, tag="bias")
            nc.sync.dma_start(bias_sb[:], bias2[:])
            if with_attn:
                cwt1 = const.tile([C, C], F32, tag="cwt1")
                nc.sync.dma_start(cwt1[:], cwt1_d[:])
                cwt2 = const.tile([C, C], F32, tag="cwt2")
                nc.sync.dma_start(cwt2[:], cwt2_d[:])
                gam = const.tile([128, 1], F32, tag="gam")
                nc.sync.dma_start(gam[:], gam_d[:])
                omg = const.tile([128, 1], F32, tag="omg")
                nc.sync.dma_start(omg[:], omg_d[:])
                ident = const.tile([128, 128], F32, tag="ident")
                nc.sync.dma_start(ident[:], ident_d[:])

            for i in range(BPC):
                # Load both maps in split layout [128, 8192]:
                # partition h*64+c <- v[c, h*8192 + n]
                # Two DMAs per map (one per half): 2D DRAM APs with outer
                # count 64 so HWDGE round-robins descriptors over all 16
                # SDMA engines (a 3D AP with outer count 2 lands on 2).
                t128 = vpool.tile([128, HALF], F32, tag="t")
                nc.sync.dma_start(t128[0:64, :], t_in[i, :, 0:HALF])
                nc.sync.dma_start(t128[64:128, :], t_in[i, :, HALF:N])
                r128 = vpool.tile([128, HALF], F32, tag="r")
                nc.sync.dma_start(r128[0:64, :], r_in[i, :, 0:HALF])
                nc.sync.dma_start(r128[64:128, :], r_in[i, :, HALF:N])

                if with_attn:
                    attn = {}
                    for name, v128 in (("t", t128), ("r", r128)):
                        # E_grand[a, b] = sum_f v128[a, f] v128[b, f], via
                        # PE-transposed chunks; E = diag-fold of E_grand.
                        eg_ps = egpool.tile([128, 128], F32, tag="eg")
                        for g in range(HALF // CK):
                            tp = tppool.tile([128, CK], F32, tag="tp")
                            for q in range(4):
                                k = 4 * g + q
                                nc.tensor.transpose(
                                    tp[:, 128 * q : 128 * (q + 1)],
                                    v128[:, 128 * k : 128 * (k + 1)],
                                    ident[:],
                                )
                            at = atpool.tile([128, CK], F32, tag="at")
                            nc.scalar.copy(at[:], tp[:])
                            for q in range(4):
                                k = 4 * g + q
                                sl = at[:, 128 * q : 128 * (q + 1)]
                                nc.tensor.matmul(
                                    eg_ps[:],
                                    sl,
                                    sl,
                                    start=(k == 0),
                                    stop=(k == HALF // 128 - 1),
                                )
                        egs = smpool.tile([128, 128], F32, tag="egs")
                        nc.vector.tensor_copy(egs[:], eg_ps[:])
                        eglow = smpool.tile([C, C], F32, tag="eglow")
                        nc.sync.dma_start(eglow[:], egs[64:128, 64:128])
                        e = smpool.tile([C, C], F32, tag="e")
                        nc.vector.tensor_add(e[:], egs[0:64, 0:64], eglow[:])
                        # softmax(rowmax(E)-E) == exp(rowmin(E)-E)/sum(...)
                        rmin = smpool.tile([C, 1], F32, tag="rmin")
                        nc.vector.tensor_reduce(
                            rmin[:], e[:], axis=mybir.AxisListType.X,
                            op=mybir.AluOpType.min,
                        )
                        p = smpool.tile([C, C], F32, tag="p")
                        rsum = smpool.tile([C, 1], F32, tag="rsum")
                        nc.scalar.activation(
                            p[:], e[:], Exp, bias=rmin[:], scale=-1.0,
                            accum_out=rsum[:],
                        )
                        rinv = smpool.tile([C, 1], F32, tag="rinv")
                        nc.vector.reciprocal(rinv[:], rsum[:])
                        a = smpool.tile([C, C], F32, tag=f"attn_{name}")
                        nc.vector.tensor_scalar_mul(a[:], p[:], rinv[:])
                        attn[name] = a

                    # W_x diag blocks: M_tT = gamma*(w1@r_attn).T + w1T, etc.
                    # (w1@r_attn).T = r_attn.T.T @ w1T = matmul(lhsT=r_attn, rhs=w1T)
                    for wtile, a, cw, g_ap in (
                        (Wt, attn["r"], cwt1, gam),
                        (Wr, attn["t"], cwt2, omg),
                    ):
                        p1 = p1pool.tile([C, C], F32, tag="p1")
                        nc.tensor.matmul(p1[:], a[:], cw[:], start=True, stop=True)
                        tmp = smpool.tile([C, C], F32, tag="tmp")
                        nc.vector.tensor_scalar_mul(tmp[:], p1[:], g_ap[0:64, :])
                        nc.vector.tensor_add(wtile[0:64, 0:64], tmp[:], cw[:])
                        nc.sync.dma_start(wtile[64:128, 64:128], wtile[0:64, 0:64])

                # out128 = Wt.T @ t128 + Wr.T @ r128 + bias (split layout)
                group = 8 if not with_attn else 4
                for g in range(NCHUNK // group):
                    pss = []
                    for q in range(group):
                        j = group * g + q
                        ps = pspool.tile([128, CK], F32, tag="ps")
                        nc.tensor.matmul(
                            ps[:], Wt[:], t128[:, CK * j : CK * (j + 1)],
                            start=True, stop=False,
                        )
                        pss.append((j, ps))
                    for j, ps in pss:
                        nc.tensor.matmul(
                            ps[:], Wr[:], r128[:, CK * j : CK * (j + 1)],
                            start=False, stop=True,
                        )
                    for j, ps in pss:
                        oc = ocpool.tile([128, CK], F32, tag="oc")
                        nc.scalar.activation(
                            oc[:], ps[:], Ident, bias=bias_sb[:], scale=1.0
                        )
                        nc.scalar.dma_start(
                            out[i, :, CK * j : CK * (j + 1)], oc[0:64, :]
                        )
                        nc.scalar.dma_start(
                            out[i, :, HALF + CK * j : HALF + CK * (j + 1)],
                            oc[64:128, :],
                        )

    nc.compile()
    return nc


def _get_program(with_attn: bool):
    prog = _programs.get(with_attn)
    if prog is None:
        prog = _build_program(with_attn)
        _programs[with_attn] = prog
    return prog


def kernel(template_map, roi_map, gamma, omega, conv_w, conv_b):
    template_map = np.ascontiguousarray(np.asarray(template_map, dtype=np.float32))
    roi_map = np.ascontiguousarray(np.asarray(roi_map, dtype=np.float32))
    conv_w = np.asarray(conv_w, dtype=np.float32)
    conv_b = np.asarray(conv_b, dtype=np.float32)
    g = float(np.asarray(gamma).reshape(-1)[0])
    o = float(np.asarray(omega).reshape(-1)[0])
    with_attn = not (g == 0.0 and o == 0.0)

    nc = _get_program(with_attn)

    w1T = np.ascontiguousarray(conv_w[:, :C].T)  # [c, o]
    w2T = np.ascontiguousarray(conv_w[:, C:].T)
    wt0 = np.zeros((128, 128), np.float32)
    wt0[:64, :64] = w1T
    wt0[64:, 64:] = w1T
    wr0 = np.zeros((128, 128), np.float32)
    wr0[:64, :64] = w2T
    wr0[64:, 64:] = w2T
    bias2 = np.ascontiguousarray(np.tile(conv_b, 2)[:, None])  # [128, 1]

    common = {"wt0": wt0, "wr0": wr0, "bias2": bias2}
    if with_attn:
        common.update(
            cwt1=w1T,
            cwt2=w2T,
            gam2=np.full((128, 1), g, np.float32),
            omg2=np.full((128, 1), o, np.float32),
            ident=np.eye(128, dtype=np.float32),
        )

    tm = template_map.reshape(B, C, N)
    rm = roi_map.reshape(B, C, N)
    in_maps = [
        dict(
            common,
            t_in=tm[BPC * i : BPC * (i + 1)],
            r_in=rm[BPC * i : BPC * (i + 1)],
        )
        for i in range(NCORES)
    ]

    res = bass_utils.run_bass_kernel_spmd(nc, in_maps, core_ids=list(range(NCORES)))
    outp = np.concatenate([res.results[i]["out"] for i in range(NCORES)], axis=0)
    return outp.reshape(B, C, H, W)
